# revision 28
# baseline (speedup 1.0000x reference)
"""Trainium2 Bass kernel for CombinedAdvancedLoss (focal + contrastive +
circularity + consensus), data-parallel over 8 NeuronCores.

v3 design:
- logits shipped fp8-e4m3 (2MB/core), everything else bf16, all
  partition-major contiguous. Host gathers the target-logit plane xt
  (pure indexing), so focal is ce = ln(sum_c exp(l_c)) - xt.
- Focal pipelined per image: exp chunk -> 8 accumulating identity matmuls
  into PSUM -> ln -> elementwise tail (2 half-chunks).
- Features shipped pre-transposed+rolled; similarity = unnormalized Gram
  (PE) scaled post-hoc by on-chip row/col inverse norms.
- Work spread across engines: scalar (exp/ln), vector (fused
  scalar_tensor_tensor product-sums, reduces), PE (channel sums, row-diff
  matmuls, column sums), gpsimd (feature squares, row diffs).
- Two DMA queues (SP + Activation) so small inputs land while logits
  stream.

Each core emits a [1,32] vector of linear partial sums; the host combines
them (IoU ratios and the circularity formula act on a handful of scalars).
"""

import sys

for _p in ("/opt/trn_rl_repo",):
    if _p not in sys.path:
        sys.path.insert(0, _p)

import numpy as np
import ml_dtypes

import concourse.bass as bass
import concourse.tile as tile
from concourse import mybir
from concourse.bass_utils import run_bass_kernel_spmd

import bass_rust as _bass_rust

# ---------------------------------------------------------------------------
# The walrus build in this container rejects >2 sync waits per instruction.
# Post-pass: hoist excess waits onto inserted same-engine NoOps.
_WAIT_CAP = 1


def _split_sync_waits(nc):
    n = 0
    for fn in nc.m.functions:
        for blk in fn.blocks:
            insts = blk.instructions
            i = 0
            while i < len(insts):
                inst = insts[i]
                si = inst.sync_info
                if si is not None and len(si.on_wait) > _WAIT_CAP:
                    waits = list(si.on_wait)
                    keep = waits[-_WAIT_CAP:]
                    extra = waits[:-_WAIT_CAP]
                    nops = []
                    for j in range(0, len(extra), _WAIT_CAP):
                        nop = mybir.InstNoOp(
                            name=f"I-wsplit-{n}", engine=inst.engine)
                        n += 1
                        nop.sync_info = _bass_rust.SyncInfo(
                            on_wait=extra[j:j + _WAIT_CAP], on_update=[])
                        nops.append(nop)
                    inst.sync_info = _bass_rust.SyncInfo(
                        on_wait=keep, on_update=list(si.on_update))
                    for k, nop in enumerate(nops):
                        insts.insert(i + k, nop)
                    i += len(nops)
                i += 1
# ---------------------------------------------------------------------------

F32 = mybir.dt.float32
BF16 = mybir.dt.bfloat16
FP8 = mybir.dt.float8e3
AF = mybir.ActivationFunctionType
OP = mybir.AluOpType
AX = mybir.AxisListType

NCORES = 8
B, C, H, W = 32, 8, 256, 256
BP = B // NCORES          # batch items per core (4)
FD = 2048                 # free dim of a full-core pixel tile (BP*512)
TEMP = 0.07
NPART = 32                # width of the per-core partials vector

# partials vector layout
K_FOCAL = 0               # 2 cols (half-chunks): sum 0.25*(1-p)^2 * ce
K_CONTRAST = 2            # sum (lse - pos) over this core's 128 rows
K_AREA = 3                # 4 cols: per-b mask area
K_EX = 7                  # 4 cols: per-b sum |dm/dh| (incl. half boundary)
K_EY = 11                 # 8 cols: per-(b, half) sum |dm/dw|
K_S = 19                  # 3 cols: per-method sum of preds
K_I = 22                  # 3 cols: per-pair sum pi*pj  (01, 02, 12)


def _build_nc():
    nc = bass.Bass()

    lg = nc.declare_dram_parameter("lg", [128, BP, C, 512], FP8, isOutput=False)
    xt = nc.declare_dram_parameter("xt", [128, BP, 512], FP8, isOutput=False)
    mp = nc.declare_dram_parameter("mp", [128, 3, BP, 512], FP8, isOutput=False)
    mk = nc.declare_dram_parameter("mk", [128, BP, 2, 256], FP8, isOutput=False)
    ft = nc.declare_dram_parameter("ft", [128, 4, 1024], FP8, isOutput=False)
    cbf = nc.declare_dram_parameter("cbf", [128, 4, 128], BF16, isOutput=False)
    cf32 = nc.declare_dram_parameter("cf32", [128, 128], F32, isOutput=False)
    out = nc.declare_dram_parameter("partials", [1, NPART], F32, isOutput=True)

    with tile.TileContext(nc) as tc:
        _emit(nc, tc, lg, xt, mp, mk, ft, cbf, cf32, out)
    _split_sync_waits(nc)
    return nc


def _emit(nc, tc, lg, xt, mp, mk, ft, cbf, cf32, out):
    from contextlib import ExitStack

    ctx = ExitStack()
    with ctx:
        singles = ctx.enter_context(tc.tile_pool(name="singles", bufs=1))
        scratch = ctx.enter_context(tc.tile_pool(name="scratch", bufs=2))
        tiny = ctx.enter_context(tc.tile_pool(name="tiny", bufs=1))
        psA = ctx.enter_context(tc.tile_pool(name="psA", bufs=1, space="PSUM"))
        psB = ctx.enter_context(tc.tile_pool(name="psB", bufs=2, space="PSUM"))
        psC = ctx.enter_context(tc.tile_pool(name="psC", bufs=1, space="PSUM"))

        # ---------------- vector-engine constants ----------------
        ones_b = singles.tile([128, 1], BF16)
        nc.vector.memset(ones_b, 1.0)
        ones_f = singles.tile([128, 1], F32)
        nc.vector.memset(ones_f, 1.0)
        ones_r = singles.tile([1, 128], F32)
        nc.vector.memset(ones_r, 1.0)
        acc = singles.tile([128, NPART], F32)
        nc.vector.memset(acc, 0.0)

        # scalar: warm the exp/ln activation-table before any data lands
        warm = tiny.tile([1, 1], F32, tag="warm")
        nc.scalar.activation(out=warm, in_=ones_f[0:1, :], func=AF.Exp)

        # ---------------- DMAs (single sync queue, by need-time) ------
        lg_t = singles.tile([128, BP, C, 512], FP8)
        cb_t = singles.tile([128, 4, 128], BF16)
        ft_t = singles.tile([128, 4, 1024], FP8)
        mk_t = singles.tile([128, BP, 2, 256], FP8)
        mp_t = singles.tile([128, 3, BP, 512], FP8)
        ident_f = singles.tile([128, 128], F32)
        xt_t = singles.tile([128, BP, 512], FP8)
        nc.sync.dma_start(out=lg_t[:, 0, 0:4], in_=lg[:, 0, 0:4])
        nc.sync.dma_start(out=lg_t[:, 0, 4:8], in_=lg[:, 0, 4:8])
        nc.sync.dma_start(out=cb_t, in_=cbf[:, :, :])
        nc.sync.dma_start(out=ft_t, in_=ft[:, :, :])
        nc.sync.dma_start(out=lg_t[:, 1], in_=lg[:, 1])
        nc.sync.dma_start(out=mk_t, in_=mk[:, :, :, :])
        nc.sync.dma_start(out=lg_t[:, 2], in_=lg[:, 2])
        nc.sync.dma_start(out=mp_t, in_=mp[:, :, :, :])
        nc.sync.dma_start(out=lg_t[:, 3], in_=lg[:, 3])
        nc.sync.dma_start(out=xt_t, in_=xt[:, :, :])
        nc.sync.dma_start(out=ident_f, in_=cf32[:, :])

        ident_b = cb_t[:, 0]

        # ---------------- feature squares (split) + row diffs ---------
        sq = singles.tile([128, 4, 1024], BF16)
        nc.vector.tensor_tensor(
            out=sq[:, 0:2].rearrange("p a b -> p (a b)"),
            in0=ft_t[:, 0:2].rearrange("p a b -> p (a b)"),
            in1=ft_t[:, 0:2].rearrange("p a b -> p (a b)"), op=OP.mult)
        nc.gpsimd.tensor_tensor(
            out=sq[:, 2:4].rearrange("p a b -> p (a b)"),
            in0=ft_t[:, 2:4].rearrange("p a b -> p (a b)"),
            in1=ft_t[:, 2:4].rearrange("p a b -> p (a b)"), op=OP.mult)
        d_y = singles.tile([128, BP, 2, 255], BF16)
        nc.gpsimd.tensor_tensor(
            out=d_y, in0=mk_t[:, :, :, 1:256], in1=mk_t[:, :, :, 0:255],
            op=OP.subtract)

        # ---------------- scalar: exp spine ---------------------------
        q = singles.tile([128, BP, C, 512], BF16)
        s_ps = psA.tile([128, 2048], F32, tag="s")

        def exp_chunk(j):
            nc.scalar.activation(
                out=q[:, j].rearrange("p c x -> p (c x)"),
                in_=lg_t[:, j].rearrange("p c x -> p (c x)"),
                func=AF.Exp)

        def smm_chunk(j):
            for c in range(C):
                nc.tensor.matmul(
                    out=s_ps[:, 512 * j:512 * (j + 1)],
                    lhsT=ident_b,
                    rhs=q[:, j, c],
                    start=(c == 0), stop=(c == C - 1))

        nc.scalar.activation(
            out=q[:, 0, 0:4].rearrange("p c x -> p (c x)"),
            in_=lg_t[:, 0, 0:4].rearrange("p c x -> p (c x)"), func=AF.Exp)
        nc.scalar.activation(
            out=q[:, 0, 4:8].rearrange("p c x -> p (c x)"),
            in_=lg_t[:, 0, 4:8].rearrange("p c x -> p (c x)"), func=AF.Exp)
        exp_chunk(1)
        exp_chunk(2)

        # contrastive norms: ss column sums on PE, logs squeezed into the
        # exp spine (deps land well before exp2 completes)
        smm_chunk(0)
        smm_chunk(1)
        ss_ps = psC.tile([1, 1024], F32, tag="g", name="ss_ps")
        for half in range(2):
            for dc in range(4):
                nc.tensor.matmul(
                    out=ss_ps[:, 512 * half:512 * (half + 1)],
                    lhsT=ones_b,
                    rhs=sq[:, dc, 512 * half:512 * (half + 1)],
                    start=(dc == 0), stop=(dc == 3))
        lnss = tiny.tile([1, 1024], F32, tag="lnss")
        nc.scalar.activation(out=lnss, in_=ss_ps, func=AF.Ln)
        colinv = tiny.tile([1, 1024], F32, tag="colinv")
        nc.scalar.activation(out=colinv, in_=lnss, func=AF.Exp, scale=-0.5)

        exp_chunk(3)
        for b in range(2, BP):
            ajs = scratch.tile([128, 512], BF16, tag="aj", name=f"ajs{b}")
            nc.scalar.activation(
                out=ajs, in_=mk_t[:, b].rearrange("p h w -> p (h w)"),
                func=AF.Copy,
                accum_out=acc[:, K_AREA + b:K_AREA + b + 1])

        # ---------------- PE program --------------------------------
        smm_chunk(2)
        g_ps = psC.tile([128, 1024], F32, tag="g", name="g_ps")
        for half in range(2):
            for dc in range(4):
                nc.tensor.matmul(
                    out=g_ps[:, 512 * half:512 * (half + 1)],
                    lhsT=ft_t[:, dc, 0:128],
                    rhs=ft_t[:, dc, 512 * half:512 * (half + 1)],
                    start=(dc == 0), stop=(dc == 3))
        # circularity row-diff matmuls (3 per image, incl. half boundary)
        for b in range(BP):
            cps = psB.tile([128, 512], F32, tag="sm", name=f"cps{b}")
            nc.tensor.matmul(
                out=cps[:, 0:256], lhsT=cb_t[:, 1], rhs=mk_t[:, b, 0],
                start=True, stop=False)
            nc.tensor.matmul(
                out=cps[:, 0:256], lhsT=cb_t[:, 2], rhs=mk_t[:, b, 1],
                start=False, stop=True)
            nc.tensor.matmul(
                out=cps[:, 256:512], lhsT=cb_t[:, 3], rhs=mk_t[:, b, 1],
                start=True, stop=True)
            nc.vector.tensor_reduce(
                out=acc[:, K_EX + b:K_EX + b + 1], in_=cps,
                axis=AX.XY, op=OP.add, apply_absolute_value=True)
        # colinv broadcast into two psB half-slots (avoids psC ring cycle)
        cbh = []
        for half in range(2):
            cbp = psB.tile([128, 512], F32, tag="sm", name=f"cbp{half}")
            nc.tensor.matmul(
                out=cbp, lhsT=ones_r,
                rhs=colinv[:, 512 * half:512 * (half + 1)],
                start=True, stop=True)
            cbh.append(cbp)
        smm_chunk(3)
        # consensus S_i column sums: ones.T @ mp chunks -> [1, 512]
        for i in range(3):
            sps = psB.tile([1, 512], F32, tag="sm", name=f"sps{i}")
            mflat = mp_t[:, i].rearrange("p b x -> p (b x)")
            for k in range(4):
                nc.tensor.matmul(
                    out=sps,
                    lhsT=ones_b,
                    rhs=mflat[:, 512 * k:512 * (k + 1)],
                    start=(k == 0), stop=(k == 3))
            sjunk = scratch.tile([1, 512], F32, tag="sjk", name=f"sjk{i}")
            nc.vector.tensor_scalar(
                out=sjunk, in0=sps, scalar1=1.0, scalar2=0.0,
                op0=OP.mult, op1=OP.add,
                accum_out=acc[0:1, K_S + i:K_S + i + 1])
        # rowinv = colinv[0:128] transposed, scaled by 1/T
        rT_ps = psB.tile([128, 1], F32, tag="sm", name="rT_ps")
        nc.tensor.transpose(
            out=rT_ps, in_=colinv[:, 0:128], identity=ident_f[0:1, 0:1])

        # ---------------- vector program ------------------------------
        for b in range(2):
            aj = scratch.tile([128, 512], BF16, tag="aj", name=f"aj{b}")
            nc.vector.tensor_scalar(
                out=aj, in0=mk_t[:, b].rearrange("p h w -> p (h w)"),
                scalar1=1.0, scalar2=0.0, op0=OP.mult, op1=OP.add,
                accum_out=acc[:, K_AREA + b:K_AREA + b + 1])
        # consensus pair intersections (fused product+sum)
        for k, (i, j) in enumerate(((0, 1), (0, 2), (1, 2))):
            ij = scratch.tile([128, 2048], BF16, tag="wj", name=f"ij{k}")
            nc.vector.scalar_tensor_tensor(
                out=ij, in0=mp_t[:, i].rearrange("p b x -> p (b x)"),
                scalar=1.0, in1=mp_t[:, j].rearrange("p b x -> p (b x)"),
                op0=OP.mult, op1=OP.mult,
                accum_out=acc[:, K_I + k:K_I + k + 1])
        # colinv broadcast to SBUF + rowinv
        colbc = singles.tile([128, 1024], F32)
        nc.vector.tensor_copy(out=colbc[:, 0:512], in_=cbh[0])
        nc.vector.tensor_copy(out=colbc[:, 512:1024], in_=cbh[1])
        rowinv = tiny.tile([128, 1], F32, tag="rowinv")
        nc.vector.tensor_scalar(
            out=rowinv, in0=rT_ps, scalar1=1.0 / TEMP, scalar2=None,
            op0=OP.mult)
        st2 = singles.tile([128, 1024], F32)
        nc.vector.tensor_tensor(out=st2, in0=g_ps, in1=colbc, op=OP.mult)
        nc.vector.scalar_tensor_tensor(
            out=st2[:, 0:128], in0=ident_f, scalar=-1e5,
            in1=st2[:, 0:128], op0=OP.mult, op1=OP.add)
        posj = scratch.tile([128, 128], F32, tag="posj")
        posr = tiny.tile([128, 1], F32, tag="posr")
        nc.vector.scalar_tensor_tensor(
            out=posj, in0=st2[:, 512:640], scalar=1.0, in1=ident_f,
            op0=OP.mult, op1=OP.mult, accum_out=posr)
        # ey: in-row diffs reduce
        nc.vector.tensor_reduce(
            out=acc[:, K_EY:K_EY + 8].rearrange("p (b c) -> p b c", b=BP),
            in_=d_y, axis=AX.X, op=OP.add, apply_absolute_value=True)

        # ---------------- focal logs + contrastive exp ----------------
        ln_s = singles.tile([128, 2048], BF16)
        p_t = singles.tile([128, 2048], BF16)
        ce = singles.tile([128, 2048], BF16)
        u_t = singles.tile([128, 2048], BF16)
        v_t = singles.tile([128, 2048], BF16)
        xtf = xt_t.rearrange("p b x -> p (b x)")
        for h in range(2):
            sl = slice(1024 * h, 1024 * (h + 1))
            nc.scalar.activation(out=ln_s[:, sl], in_=s_ps[:, sl], func=AF.Ln)
        esim = scratch.tile([128, 1024], BF16, tag="esim")
        rsum = tiny.tile([128, 1], F32, tag="rsum")
        nc.scalar.activation(
            out=esim, in_=st2, func=AF.Exp, scale=rowinv, accum_out=rsum)
        lse = tiny.tile([128, 1], F32, tag="lse")
        nc.scalar.activation(out=lse, in_=rsum, func=AF.Ln)
        # contrast partial: lse - pos*rowinv
        post = tiny.tile([128, 1], F32, tag="post")
        nc.vector.tensor_scalar(
            out=post, in0=posr, scalar1=rowinv, scalar2=None, op0=OP.mult)
        nc.vector.tensor_tensor(
            out=acc[:, K_CONTRAST:K_CONTRAST + 1], in0=lse, in1=post,
            op=OP.subtract)
        for h in range(2):
            sl = slice(1024 * h, 1024 * (h + 1))
            nc.vector.tensor_tensor(
                out=ce[:, sl], in0=ln_s[:, sl], in1=xtf[:, sl],
                op=OP.subtract)
        for h in range(2):
            sl = slice(1024 * h, 1024 * (h + 1))
            nc.scalar.activation(
                out=p_t[:, sl], in_=ce[:, sl], func=AF.Exp, scale=-1.0)
        for h in range(2):
            sl = slice(1024 * h, 1024 * (h + 1))
            nc.vector.tensor_scalar(
                out=u_t[:, sl], in0=p_t[:, sl], scalar1=-1.0, scalar2=1.0,
                op0=OP.mult, op1=OP.add)
            nc.vector.tensor_tensor(
                out=v_t[:, sl], in0=u_t[:, sl], in1=u_t[:, sl], op=OP.mult)
            wj = scratch.tile([128, 1024], BF16, tag="wj2", name=f"wj{h}")
            nc.vector.scalar_tensor_tensor(
                out=wj, in0=v_t[:, sl], scalar=0.25, in1=ce[:, sl],
                op0=OP.mult, op1=OP.mult,
                accum_out=acc[:, K_FOCAL + h:K_FOCAL + h + 1])

        # ---------------- partition-reduce + store --------------------
        pfin = psB.tile([1, NPART], F32, tag="sm", name="pfin")
        nc.tensor.matmul(out=pfin, lhsT=ones_f, rhs=acc, start=True, stop=True)
        out_t = tiny.tile([1, NPART], F32, tag="outt")
        nc.vector.tensor_copy(out=out_t, in_=pfin)
        nc.sync.dma_start(out=out[:, :], in_=out_t)


def _zmats():
    """lhsT matrices for row-diff matmuls: out[r] = sum_p Z[p, r] * m[p]."""
    zmA = np.zeros((128, 128), dtype=np.float32)
    zmB = np.zeros((128, 128), dtype=np.float32)
    zmC = np.zeros((128, 128), dtype=np.float32)
    for r in range(127):
        zmA[r + 1, r] = 1.0
        zmA[r, r] = -1.0
        zmC[r + 1, r] = 1.0
        zmC[r, r] = -1.0
    zmA[127, 127] = -1.0   # half0 row127: -m0[127], completed by zmB
    zmB[0, 127] = 1.0      # + m1[0]  -> cross-half boundary diff
    return zmA, zmB, zmC


def _host_inputs(logits, target, features, masks, method_preds):
    """Slice/reshape/convert full inputs into per-core input maps."""
    bf = ml_dtypes.bfloat16
    f8 = ml_dtypes.float8_e3m4
    ident = np.eye(128, dtype=np.float32)
    zmA, zmB, zmC = _zmats()
    cbf = np.ascontiguousarray(
        np.stack([ident, zmA, zmB, zmC], axis=1).astype(bf))  # [128,4,128]
    consts = {"cbf": cbf, "cf32": ident}
    # gather target logit plane on host (pure indexing)
    xt_full = np.take_along_axis(
        logits, target[:, None].astype(np.int64), axis=1)[:, 0]  # [B, H, W]
    in_maps = []
    for c in range(NCORES):
        b0 = c * BP
        lgs = logits[b0:b0 + BP]                                # [4,8,256,256]
        lg_pm = lgs.reshape(BP, C, 128, 512).transpose(2, 0, 1, 3)
        xt_pm = xt_full[b0:b0 + BP].reshape(BP, 128, 512).transpose(1, 0, 2)
        mp_pm = method_preds[:, b0:b0 + BP].reshape(
            3, BP, 128, 512).transpose(2, 0, 1, 3)
        mk_pm = masks[b0:b0 + BP, 0].reshape(BP, 2, 128, 256).transpose(
            2, 0, 1, 3)
        fr = np.roll(features, -c * 128, axis=0)                # [1024, 512]
        ft_pm = fr.T.reshape(4, 128, 1024).transpose(1, 0, 2)
        in_maps.append({
            "lg": np.ascontiguousarray(lg_pm.astype(f8)),
            "xt": np.ascontiguousarray(xt_pm.astype(f8)),
            "mp": np.ascontiguousarray(mp_pm.astype(f8)),
            "mk": np.ascontiguousarray(mk_pm.astype(f8)),
            "ft": np.ascontiguousarray(ft_pm.astype(f8)),
            **consts,
        })
    return in_maps


def _combine(partials):
    """Host-side combination of the per-core [1,32] partial vectors."""
    P = np.stack([np.asarray(p).reshape(-1).astype(np.float64)
                  for p in partials])  # [8,32]
    HW = H * W
    focal = (P[:, K_FOCAL] + P[:, K_FOCAL + 1]).sum() / (B * HW)
    contrast = 0.5 * P[:, K_CONTRAST].sum() / 1024

    circ_total = 0.0
    for c in range(NCORES):
        for b in range(BP):
            area = P[c, K_AREA + b]
            ex = P[c, K_EX + b]
            ey = P[c, K_EY + 2 * b] + P[c, K_EY + 2 * b + 1]
            per = ex + ey
            if area > 0 and per > 0:
                circv = 4.0 * np.pi * area / max(per, 1e-12) ** 2
                circ_total += (circv - 1.0) ** 2
    circ = 0.1 * circ_total / B

    S = P[:, K_S:K_S + 3].sum(axis=0)
    I = P[:, K_I:K_I + 3].sum(axis=0)
    cons_total = 0.0
    for k, (i, j) in enumerate(((0, 1), (0, 2), (1, 2))):
        union = S[i] + S[j] - I[k]
        iou = I[k] / (union + 1e-6)
        cons_total += max(0.6 - iou, 0.0)
    consensus = 0.3 * cons_total / 3.0

    return np.float32(focal + contrast + circ + consensus)


_CACHED_NC = None


def _get_nc():
    global _CACHED_NC
    if _CACHED_NC is None:
        _CACHED_NC = _build_nc()
    return _CACHED_NC


def kernel(logits, target, features, masks, method_preds):
    logits = np.asarray(logits, dtype=np.float32)
    target = np.asarray(target, dtype=np.int32)
    features = np.asarray(features, dtype=np.float32)
    masks = np.asarray(masks, dtype=np.float32)
    method_preds = np.asarray(method_preds, dtype=np.float32)

    in_maps = _host_inputs(logits, target, features, masks, method_preds)
    res = run_bass_kernel_spmd(_get_nc(), in_maps, list(range(NCORES)))
    partials = [res.results[c]["partials"] for c in range(NCORES)]
    return _combine(partials)


# revision 30
# speedup vs baseline: 1.0699x; 1.0699x over previous
"""Trainium2 Bass kernel for CombinedAdvancedLoss (focal + contrastive +
circularity + consensus), data-parallel over 8 NeuronCores.

v3 design:
- logits shipped fp8-e4m3 (2MB/core), everything else bf16, all
  partition-major contiguous. Host gathers the target-logit plane xt
  (pure indexing), so focal is ce = ln(sum_c exp(l_c)) - xt.
- Focal pipelined per image: exp chunk -> 8 accumulating identity matmuls
  into PSUM -> ln -> elementwise tail (2 half-chunks).
- Features shipped pre-transposed+rolled; similarity = unnormalized Gram
  (PE) scaled post-hoc by on-chip row/col inverse norms.
- Work spread across engines: scalar (exp/ln), vector (fused
  scalar_tensor_tensor product-sums, reduces), PE (channel sums, row-diff
  matmuls, column sums), gpsimd (feature squares, row diffs).
- Two DMA queues (SP + Activation) so small inputs land while logits
  stream.

Each core emits a [1,32] vector of linear partial sums; the host combines
them (IoU ratios and the circularity formula act on a handful of scalars).
"""

import sys

for _p in ("/opt/trn_rl_repo",):
    if _p not in sys.path:
        sys.path.insert(0, _p)

import numpy as np
import ml_dtypes

import concourse.bass as bass
import concourse.tile as tile
from concourse import mybir
from concourse.bass_utils import run_bass_kernel_spmd

import bass_rust as _bass_rust

# ---------------------------------------------------------------------------
# The walrus build in this container rejects >2 sync waits per instruction.
# Post-pass: hoist excess waits onto inserted same-engine NoOps.
_WAIT_CAP = 1


def _split_sync_waits(nc):
    n = 0
    for fn in nc.m.functions:
        for blk in fn.blocks:
            insts = blk.instructions
            i = 0
            while i < len(insts):
                inst = insts[i]
                si = inst.sync_info
                if si is not None and len(si.on_wait) > _WAIT_CAP:
                    waits = list(si.on_wait)
                    keep = waits[-_WAIT_CAP:]
                    extra = waits[:-_WAIT_CAP]
                    nops = []
                    for j in range(0, len(extra), _WAIT_CAP):
                        nop = mybir.InstNoOp(
                            name=f"I-wsplit-{n}", engine=inst.engine)
                        n += 1
                        nop.sync_info = _bass_rust.SyncInfo(
                            on_wait=extra[j:j + _WAIT_CAP], on_update=[])
                        nops.append(nop)
                    inst.sync_info = _bass_rust.SyncInfo(
                        on_wait=keep, on_update=list(si.on_update))
                    for k, nop in enumerate(nops):
                        insts.insert(i + k, nop)
                    i += len(nops)
                i += 1
# ---------------------------------------------------------------------------

F32 = mybir.dt.float32
BF16 = mybir.dt.bfloat16
FP8 = mybir.dt.float8e3
AF = mybir.ActivationFunctionType
OP = mybir.AluOpType
AX = mybir.AxisListType

NCORES = 8
B, C, H, W = 32, 8, 256, 256
BP = B // NCORES          # batch items per core (4)
FD = 2048                 # free dim of a full-core pixel tile (BP*512)
TEMP = 0.07
NPART = 32                # width of the per-core partials vector

# partials vector layout
K_FOCAL = 0               # 2 cols (half-chunks): sum 0.25*(1-p)^2 * ce
K_CONTRAST = 2            # sum (lse - pos) over this core's 128 rows
K_AREA = 3                # 4 cols: per-b mask area
K_EX = 7                  # 4 cols: per-b sum |dm/dh| (incl. half boundary)
K_EY = 11                 # 8 cols: per-(b, half) sum |dm/dw|
K_S = 19                  # 3 cols: per-method sum of preds
K_I = 22                  # 3 cols: per-pair sum pi*pj  (01, 02, 12)


def _build_nc():
    nc = bass.Bass()

    lg = nc.declare_dram_parameter("lg", [128, BP, C, 512], FP8, isOutput=False)
    xt = nc.declare_dram_parameter("xt", [128, BP, 512], BF16, isOutput=False)
    mp = nc.declare_dram_parameter("mp", [128, 3, BP, 512], FP8, isOutput=False)
    mk = nc.declare_dram_parameter("mk", [128, BP, 2, 256], FP8, isOutput=False)
    ft = nc.declare_dram_parameter("ft", [128, 4, 1024], FP8, isOutput=False)
    cbf = nc.declare_dram_parameter("cbf", [128, 4, 128], BF16, isOutput=False)
    cf32 = nc.declare_dram_parameter("cf32", [128, 128], F32, isOutput=False)
    out = nc.declare_dram_parameter("partials", [1, NPART], F32, isOutput=True)

    with tile.TileContext(nc) as tc:
        _emit(nc, tc, lg, xt, mp, mk, ft, cbf, cf32, out)
    _split_sync_waits(nc)
    return nc


def _emit(nc, tc, lg, xt, mp, mk, ft, cbf, cf32, out):
    from contextlib import ExitStack

    ctx = ExitStack()
    with ctx:
        singles = ctx.enter_context(tc.tile_pool(name="singles", bufs=1))
        scratch = ctx.enter_context(tc.tile_pool(name="scratch", bufs=2))
        tiny = ctx.enter_context(tc.tile_pool(name="tiny", bufs=1))
        psA = ctx.enter_context(tc.tile_pool(name="psA", bufs=1, space="PSUM"))
        psB = ctx.enter_context(tc.tile_pool(name="psB", bufs=2, space="PSUM"))
        psC = ctx.enter_context(tc.tile_pool(name="psC", bufs=1, space="PSUM"))

        # ---------------- vector-engine constants ----------------
        ones_b = singles.tile([128, 1], BF16)
        nc.vector.memset(ones_b, 1.0)
        ones_f = singles.tile([128, 1], F32)
        nc.vector.memset(ones_f, 1.0)
        ones_r = singles.tile([1, 128], F32)
        nc.vector.memset(ones_r, 1.0)
        acc = singles.tile([128, NPART], F32)
        nc.vector.memset(acc, 0.0)

        # scalar: warm the exp/ln activation-table before any data lands
        warm = tiny.tile([1, 1], F32, tag="warm")
        nc.scalar.activation(out=warm, in_=ones_f[0:1, :], func=AF.Exp)

        # ---------------- DMAs (single sync queue, by need-time) ------
        lg_t = singles.tile([128, BP, C, 512], FP8)
        cb_t = singles.tile([128, 4, 128], BF16)
        ft_t = singles.tile([128, 4, 1024], FP8)
        mk_t = singles.tile([128, BP, 2, 256], FP8)
        mp_t = singles.tile([128, 3, BP, 512], FP8)
        ident_f = singles.tile([128, 128], F32)
        xt_t = singles.tile([128, BP, 512], BF16)
        nc.sync.dma_start(out=cb_t, in_=cbf[:, :, :])
        nc.sync.dma_start(out=ft_t, in_=ft[:, :, :])
        nc.sync.dma_start(out=lg_t[:, 0, 0:4], in_=lg[:, 0, 0:4])
        nc.sync.dma_start(out=lg_t[:, 0, 4:8], in_=lg[:, 0, 4:8])
        nc.sync.dma_start(out=lg_t[:, 1], in_=lg[:, 1])
        nc.sync.dma_start(out=mk_t, in_=mk[:, :, :, :])
        nc.sync.dma_start(out=mp_t[:, 0], in_=mp[:, 0])
        nc.sync.dma_start(out=mp_t[:, 1], in_=mp[:, 1])
        nc.sync.dma_start(out=lg_t[:, 2], in_=lg[:, 2])
        nc.sync.dma_start(out=mp_t[:, 2], in_=mp[:, 2])
        nc.sync.dma_start(out=lg_t[:, 3], in_=lg[:, 3])
        nc.sync.dma_start(out=xt_t, in_=xt[:, :, :])
        nc.sync.dma_start(out=ident_f, in_=cf32[:, :])

        ident_b = cb_t[:, 0]

        # ---------------- gpsimd program ------------------------------
        sq = singles.tile([128, 4, 1024], BF16)
        nc.vector.tensor_tensor(
            out=sq[:, 0].rearrange("p b -> p b"),
            in0=ft_t[:, 0], in1=ft_t[:, 0], op=OP.mult)
        nc.gpsimd.tensor_tensor(
            out=sq[:, 1:4].rearrange("p a b -> p (a b)"),
            in0=ft_t[:, 1:4].rearrange("p a b -> p (a b)"),
            in1=ft_t[:, 1:4].rearrange("p a b -> p (a b)"), op=OP.mult)
        ij01 = singles.tile([128, 2048], BF16)
        nc.gpsimd.tensor_tensor(
            out=ij01, in0=mp_t[:, 0].rearrange("p b x -> p (b x)"),
            in1=mp_t[:, 1].rearrange("p b x -> p (b x)"), op=OP.mult)
        d_y = singles.tile([128, BP, 2, 255], BF16)
        nc.gpsimd.tensor_tensor(
            out=d_y, in0=mk_t[:, :, :, 1:256], in1=mk_t[:, :, :, 0:255],
            op=OP.subtract)

        # ---------------- scalar: exp spine ---------------------------
        q = singles.tile([128, BP, C, 512], BF16)
        s_psA = psA.tile([128, 1024], F32, tag="sA", name="s_psA")
        s_psB = psA.tile([128, 1024], F32, tag="sB", name="s_psB")
        s_half = [s_psA, s_psB]

        def exp_chunk(j):
            nc.scalar.activation(
                out=q[:, j].rearrange("p c x -> p (c x)"),
                in_=lg_t[:, j].rearrange("p c x -> p (c x)"),
                func=AF.Exp)

        def smm_chunk(j):
            sp = s_half[j // 2]
            off = 512 * (j % 2)
            for c in range(C):
                nc.tensor.matmul(
                    out=sp[:, off:off + 512],
                    lhsT=ident_b,
                    rhs=q[:, j, c],
                    start=(c == 0), stop=(c == C - 1))

        nc.scalar.activation(
            out=q[:, 0, 0:4].rearrange("p c x -> p (c x)"),
            in_=lg_t[:, 0, 0:4].rearrange("p c x -> p (c x)"), func=AF.Exp)
        nc.scalar.activation(
            out=q[:, 0, 4:8].rearrange("p c x -> p (c x)"),
            in_=lg_t[:, 0, 4:8].rearrange("p c x -> p (c x)"), func=AF.Exp)
        exp_chunk(1)
        exp_chunk(2)

        # contrastive norms: ss column sums on PE, logs squeezed into the
        # exp spine (deps land well before exp2 completes)
        smm_chunk(0)
        ss_ps = psC.tile([1, 1024], F32, tag="g", name="ss_ps")
        for half in range(2):
            for dc in range(4):
                nc.tensor.matmul(
                    out=ss_ps[:, 512 * half:512 * (half + 1)],
                    lhsT=ones_b,
                    rhs=sq[:, dc, 512 * half:512 * (half + 1)],
                    start=(dc == 0), stop=(dc == 3))
        lnss = tiny.tile([1, 1024], F32, tag="lnss")
        nc.scalar.activation(out=lnss, in_=ss_ps, func=AF.Ln)
        colinv = tiny.tile([1, 1024], F32, tag="colinv")
        nc.scalar.activation(out=colinv, in_=lnss, func=AF.Exp, scale=-0.5)

        exp_chunk(3)

        # ---------------- PE program --------------------------------
        smm_chunk(1)
        g_ps = psC.tile([128, 1024], F32, tag="g", name="g_ps")
        for half in range(2):
            for dc in range(4):
                nc.tensor.matmul(
                    out=g_ps[:, 512 * half:512 * (half + 1)],
                    lhsT=ft_t[:, dc, 0:128],
                    rhs=ft_t[:, dc, 512 * half:512 * (half + 1)],
                    start=(dc == 0), stop=(dc == 3))
        # circularity row-diff matmuls (3 per image, incl. half boundary)
        for b in range(BP):
            cps = psB.tile([128, 512], F32, tag="sm", name=f"cps{b}")
            nc.tensor.matmul(
                out=cps[:, 0:256], lhsT=cb_t[:, 1], rhs=mk_t[:, b, 0],
                start=True, stop=False)
            nc.tensor.matmul(
                out=cps[:, 0:256], lhsT=cb_t[:, 2], rhs=mk_t[:, b, 1],
                start=False, stop=True)
            nc.tensor.matmul(
                out=cps[:, 256:512], lhsT=cb_t[:, 3], rhs=mk_t[:, b, 1],
                start=True, stop=True)
            nc.vector.tensor_reduce(
                out=acc[:, K_EX + b:K_EX + b + 1], in_=cps,
                axis=AX.XY, op=OP.add, apply_absolute_value=True)
        # colinv broadcast into two psB half-slots (avoids psC ring cycle)
        cbh = []
        for half in range(2):
            cbp = psB.tile([128, 512], F32, tag="sm", name=f"cbp{half}")
            nc.tensor.matmul(
                out=cbp, lhsT=ones_r,
                rhs=colinv[:, 512 * half:512 * (half + 1)],
                start=True, stop=True)
            cbh.append(cbp)
        smm_chunk(2)
        # consensus S_i column sums + I01 product sum (PE + tiny accums)
        for i in range(3):
            sps = psB.tile([1, 512], F32, tag="sm", name=f"sps{i}")
            mflat = mp_t[:, i].rearrange("p b x -> p (b x)")
            for k in range(4):
                nc.tensor.matmul(
                    out=sps,
                    lhsT=ones_b,
                    rhs=mflat[:, 512 * k:512 * (k + 1)],
                    start=(k == 0), stop=(k == 3))
            sjunk = scratch.tile([1, 512], F32, tag="sjk", name=f"sjk{i}")
            nc.vector.tensor_scalar(
                out=sjunk, in0=sps, scalar1=1.0, scalar2=0.0,
                op0=OP.mult, op1=OP.add,
                accum_out=acc[0:1, K_S + i:K_S + i + 1])
        ips = psB.tile([1, 512], F32, tag="sm", name="ips")
        for k in range(4):
            nc.tensor.matmul(
                out=ips, lhsT=ones_b, rhs=ij01[:, 512 * k:512 * (k + 1)],
                start=(k == 0), stop=(k == 3))
        ijunk = scratch.tile([1, 512], F32, tag="sjk", name="ijunk")
        nc.vector.tensor_scalar(
            out=ijunk, in0=ips, scalar1=1.0, scalar2=0.0,
            op0=OP.mult, op1=OP.add,
            accum_out=acc[0:1, K_I:K_I + 1])
        smm_chunk(3)
        # rowinv = colinv[0:128] transposed, scaled by 1/T
        rT_ps = psB.tile([128, 1], F32, tag="sm", name="rT_ps")
        nc.tensor.transpose(
            out=rT_ps, in_=colinv[:, 0:128], identity=ident_f[0:1, 0:1])

        # ---------------- vector program ------------------------------
        for b in range(BP):
            aj = scratch.tile([128, 512], BF16, tag="aj", name=f"aj{b}")
            nc.vector.tensor_scalar(
                out=aj, in0=mk_t[:, b].rearrange("p h w -> p (h w)"),
                scalar1=1.0, scalar2=0.0, op0=OP.mult, op1=OP.add,
                accum_out=acc[:, K_AREA + b:K_AREA + b + 1])
        # consensus pair intersections (fused product+sum); I01 done on gp+PE
        for k, (i, j) in ((1, (0, 2)), (2, (1, 2))):
            ij = scratch.tile([128, 2048], BF16, tag="wj", name=f"ij{k}")
            nc.vector.scalar_tensor_tensor(
                out=ij, in0=mp_t[:, i].rearrange("p b x -> p (b x)"),
                scalar=1.0, in1=mp_t[:, j].rearrange("p b x -> p (b x)"),
                op0=OP.mult, op1=OP.mult,
                accum_out=acc[:, K_I + k:K_I + k + 1])
        # colinv broadcast to SBUF + rowinv
        colbc = singles.tile([128, 1024], F32)
        nc.vector.tensor_copy(out=colbc[:, 0:512], in_=cbh[0])
        nc.vector.tensor_copy(out=colbc[:, 512:1024], in_=cbh[1])
        rowinv = tiny.tile([128, 1], F32, tag="rowinv")
        nc.vector.tensor_scalar(
            out=rowinv, in0=rT_ps, scalar1=1.0 / TEMP, scalar2=None,
            op0=OP.mult)
        st2 = singles.tile([128, 1024], F32)
        nc.vector.tensor_tensor(out=st2, in0=g_ps, in1=colbc, op=OP.mult)
        nc.vector.scalar_tensor_tensor(
            out=st2[:, 0:128], in0=ident_f, scalar=-1e5,
            in1=st2[:, 0:128], op0=OP.mult, op1=OP.add)
        posj = scratch.tile([128, 128], F32, tag="posj")
        posr = tiny.tile([128, 1], F32, tag="posr")
        nc.vector.scalar_tensor_tensor(
            out=posj, in0=st2[:, 512:640], scalar=1.0, in1=ident_f,
            op0=OP.mult, op1=OP.mult, accum_out=posr)
        # ey: in-row diffs reduce
        nc.vector.tensor_reduce(
            out=acc[:, K_EY:K_EY + 8].rearrange("p (b c) -> p b c", b=BP),
            in_=d_y, axis=AX.X, op=OP.add, apply_absolute_value=True)

        # ---------------- focal logs + contrastive exp ----------------
        ln_s = singles.tile([128, 2048], BF16)
        p_t = singles.tile([128, 2048], BF16)
        ce = singles.tile([128, 2048], BF16)
        u_t = singles.tile([128, 2048], BF16)
        v_t = singles.tile([128, 2048], BF16)
        xtf = xt_t.rearrange("p b x -> p (b x)")
        for h in range(2):
            sl = slice(1024 * h, 1024 * (h + 1))
            nc.scalar.activation(out=ln_s[:, sl], in_=s_half[h], func=AF.Ln)
        esim = scratch.tile([128, 1024], BF16, tag="esim")
        rsum = tiny.tile([128, 1], F32, tag="rsum")
        nc.scalar.activation(
            out=esim, in_=st2, func=AF.Exp, scale=rowinv, accum_out=rsum)
        lse = tiny.tile([128, 1], F32, tag="lse")
        nc.scalar.activation(out=lse, in_=rsum, func=AF.Ln)
        # contrast partial: lse - pos*rowinv
        post = tiny.tile([128, 1], F32, tag="post")
        nc.vector.tensor_scalar(
            out=post, in0=posr, scalar1=rowinv, scalar2=None, op0=OP.mult)
        nc.vector.tensor_tensor(
            out=acc[:, K_CONTRAST:K_CONTRAST + 1], in0=lse, in1=post,
            op=OP.subtract)
        for h in range(2):
            sl = slice(1024 * h, 1024 * (h + 1))
            nc.vector.tensor_tensor(
                out=ce[:, sl], in0=ln_s[:, sl], in1=xtf[:, sl],
                op=OP.subtract)
        for h in range(2):
            sl = slice(1024 * h, 1024 * (h + 1))
            nc.scalar.activation(
                out=p_t[:, sl], in_=ce[:, sl], func=AF.Exp, scale=-1.0)
        for h in range(2):
            sl = slice(1024 * h, 1024 * (h + 1))
            nc.vector.tensor_scalar(
                out=u_t[:, sl], in0=p_t[:, sl], scalar1=-1.0, scalar2=1.0,
                op0=OP.mult, op1=OP.add)
            nc.vector.tensor_tensor(
                out=v_t[:, sl], in0=u_t[:, sl], in1=u_t[:, sl], op=OP.mult)
            wj = scratch.tile([128, 1024], BF16, tag="wj2", name=f"wj{h}")
            nc.vector.scalar_tensor_tensor(
                out=wj, in0=v_t[:, sl], scalar=0.25, in1=ce[:, sl],
                op0=OP.mult, op1=OP.mult,
                accum_out=acc[:, K_FOCAL + h:K_FOCAL + h + 1])

        # ---------------- partition-reduce + store --------------------
        pfin = psB.tile([1, NPART], F32, tag="sm", name="pfin")
        nc.tensor.matmul(out=pfin, lhsT=ones_f, rhs=acc, start=True, stop=True)
        out_t = tiny.tile([1, NPART], F32, tag="outt")
        nc.vector.tensor_copy(out=out_t, in_=pfin)
        nc.sync.dma_start(out=out[:, :], in_=out_t)


def _zmats():
    """lhsT matrices for row-diff matmuls: out[r] = sum_p Z[p, r] * m[p]."""
    zmA = np.zeros((128, 128), dtype=np.float32)
    zmB = np.zeros((128, 128), dtype=np.float32)
    zmC = np.zeros((128, 128), dtype=np.float32)
    for r in range(127):
        zmA[r + 1, r] = 1.0
        zmA[r, r] = -1.0
        zmC[r + 1, r] = 1.0
        zmC[r, r] = -1.0
    zmA[127, 127] = -1.0   # half0 row127: -m0[127], completed by zmB
    zmB[0, 127] = 1.0      # + m1[0]  -> cross-half boundary diff
    return zmA, zmB, zmC


def _host_inputs(logits, target, features, masks, method_preds):
    """Slice/reshape/convert full inputs into per-core input maps."""
    bf = ml_dtypes.bfloat16
    f8 = ml_dtypes.float8_e3m4
    ident = np.eye(128, dtype=np.float32)
    zmA, zmB, zmC = _zmats()
    cbf = np.ascontiguousarray(
        np.stack([ident, zmA, zmB, zmC], axis=1).astype(bf))  # [128,4,128]
    consts = {"cbf": cbf, "cf32": ident}
    # gather target logit plane on host (pure indexing)
    xt_full = np.take_along_axis(
        logits, target[:, None].astype(np.int64), axis=1)[:, 0]  # [B, H, W]
    in_maps = []
    for c in range(NCORES):
        b0 = c * BP
        lgs = logits[b0:b0 + BP]                                # [4,8,256,256]
        lg_pm = lgs.reshape(BP, C, 128, 512).transpose(2, 0, 1, 3)
        xt_pm = xt_full[b0:b0 + BP].reshape(BP, 128, 512).transpose(1, 0, 2)
        mp_pm = method_preds[:, b0:b0 + BP].reshape(
            3, BP, 128, 512).transpose(2, 0, 1, 3)
        mk_pm = masks[b0:b0 + BP, 0].reshape(BP, 2, 128, 256).transpose(
            2, 0, 1, 3)
        fr = np.roll(features, -c * 128, axis=0)                # [1024, 512]
        ft_pm = fr.T.reshape(4, 128, 1024).transpose(1, 0, 2)
        in_maps.append({
            "lg": np.ascontiguousarray(lg_pm.astype(f8)),
            "xt": np.ascontiguousarray(xt_pm.astype(bf)),
            "mp": np.ascontiguousarray(mp_pm.astype(f8)),
            "mk": np.ascontiguousarray(mk_pm.astype(f8)),
            "ft": np.ascontiguousarray(ft_pm.astype(f8)),
            **consts,
        })
    return in_maps


def _combine(partials):
    """Host-side combination of the per-core [1,32] partial vectors."""
    P = np.stack([np.asarray(p).reshape(-1).astype(np.float64)
                  for p in partials])  # [8,32]
    HW = H * W
    focal = (P[:, K_FOCAL] + P[:, K_FOCAL + 1]).sum() / (B * HW)
    contrast = 0.5 * P[:, K_CONTRAST].sum() / 1024

    circ_total = 0.0
    for c in range(NCORES):
        for b in range(BP):
            area = P[c, K_AREA + b]
            ex = P[c, K_EX + b]
            ey = P[c, K_EY + 2 * b] + P[c, K_EY + 2 * b + 1]
            per = ex + ey
            if area > 0 and per > 0:
                circv = 4.0 * np.pi * area / max(per, 1e-12) ** 2
                circ_total += (circv - 1.0) ** 2
    circ = 0.1 * circ_total / B

    S = P[:, K_S:K_S + 3].sum(axis=0)
    I = P[:, K_I:K_I + 3].sum(axis=0)
    cons_total = 0.0
    for k, (i, j) in enumerate(((0, 1), (0, 2), (1, 2))):
        union = S[i] + S[j] - I[k]
        iou = I[k] / (union + 1e-6)
        cons_total += max(0.6 - iou, 0.0)
    consensus = 0.3 * cons_total / 3.0

    return np.float32(focal + contrast + circ + consensus)


_CACHED_NC = None


def _get_nc():
    global _CACHED_NC
    if _CACHED_NC is None:
        _CACHED_NC = _build_nc()
    return _CACHED_NC


def kernel(logits, target, features, masks, method_preds):
    logits = np.asarray(logits, dtype=np.float32)
    target = np.asarray(target, dtype=np.int32)
    features = np.asarray(features, dtype=np.float32)
    masks = np.asarray(masks, dtype=np.float32)
    method_preds = np.asarray(method_preds, dtype=np.float32)

    in_maps = _host_inputs(logits, target, features, masks, method_preds)
    res = run_bass_kernel_spmd(_get_nc(), in_maps, list(range(NCORES)))
    partials = [res.results[c]["partials"] for c in range(NCORES)]
    return _combine(partials)
